# revision 37
# baseline (speedup 1.0000x reference)
"""Trainium2 Bass kernel for nn_AttnMech (sparse_attention, no-softmax attention).

Math (reference):
  q/k/v = 2x2-stride-2 convs of pose/app_pose/app  -> [B, 4*64, 48, 48]
  attn  = (Q^T K)/8 (no softmax);  out = attn @ V^T
  out   = gamma_h * out; nearest-upsample 2x; concat with pose; 1x1 conv.

Key algebraic restructure (linear attention => reassociate):
  out_h = V_h (Q_h^T K_h)^T / 8 = (V_h K_h^T) Q_h / 8 = G_h Q_h / 8
with G_h = V_h K_h^T a tiny 64x64 Gram matrix.  The per-head projection,
upsample and final 1x1 conv then fold into:
  final = fw1 @ pose_enc + up2x( W_cat @ Q + fb ) ,
  W_cat[:, 64h:64h+64] = (gamma_h/8) * fw2_h @ G_h
so the huge [2304,2304] attention matrices never exist.

Sharding over 8 cores: core c = (image b = c//2, spatial half = c%2).
Each core convs its half of the image; partial Gram matrices are
AllReduced across the core pair (64 KB); everything else is local.
All matmuls run as float32r (full PE rate for free-dim >= 256).

Implementation notes:
  - All convs keep weights as the stationary operand (single free dim as
    the hardware requires) and stream the image with multi-dim strided
    moving APs.  K/V results are then PE-transposed per 128-pixel chunk
    and immediately consumed by the Gram accumulation.
  - All constants ship in one packed [128, WLEN] DRAM blob (one DMA,
    one descriptor run per partition); image tensors are laid out
    host-side so every DMA is one contiguous run per partition.
  - The Q conv is scheduled after the AllReduce launch so the PE has
    work while the collective's ~20us fixed latency elapses.
"""

import os
import sys

for _p in ("/opt/trn_rl_repo", "/root/.axon_site/_ro/trn_rl_repo"):
    if os.path.isdir(_p) and _p not in sys.path:
        sys.path.insert(0, _p)

import numpy as np

import concourse.mybir as mybir
import concourse.tile as tile
from concourse import bacc, bass2jax

F32 = mybir.dt.float32
F32R = mybir.dt.float32r
BF16 = mybir.dt.bfloat16
ADD = mybir.AluOpType.add
IDENT = mybir.ActivationFunctionType.Identity

P = 128          # partitions
C = 256          # channels
W_IMG = 96       # full-res width
RH = 48          # rows per half (full-res)
FR = RH * W_IMG  # 4608 flat half-image
NI = 24          # local downsampled rows
NJ = 48          # downsampled cols
NLOC = NI * NJ   # 1152 local attn pixels
NT = 3           # conv free tiles of 384 (16 full-res rows each)
TW = 384
SLEN = 2 * 16 * W_IMG  # strip len per partition (both ic chunks) 3072
NMT = 9          # 128-pixel chunks of the local grid
OT = 12          # output assembly tiles of 384 (4 full-res rows)

# wpack layout (per partition, bf16 words) + separate fp32 bias blob
QW_O, KW_O, VW_O = 0, 2048, 4096
FW1_O, FW2_O = 6144, 6656
ID_O = 7168
WLEN = 7296
QB_O, FB_O, KB_O, VB_O = 0, 2, 4, 6
WSLEN = 8

_CACHED_NC = None
_RUNNER = None


def _make_runner(nc, n_cores=8):
    """Like bass2jax.run_bass_via_pjrt, but inputs are pre-placed on the
    devices (parallel transfer + aligned core start) and the jitted
    executable is cached across calls."""
    import jax
    from jax.experimental.shard_map import shard_map
    from jax.sharding import Mesh, NamedSharding, PartitionSpec

    bass2jax.install_neuronx_cc_hook()

    partition_name = (
        nc.partition_id_tensor.name if nc.partition_id_tensor else None
    )
    in_names, out_names, out_avals = [], [], []
    for alloc in nc.m.functions[0].allocations:
        if not isinstance(alloc, mybir.MemoryLocationSet):
            continue
        name = alloc.memorylocations[0].name
        if alloc.kind == "ExternalInput":
            if name != partition_name:
                in_names.append(name)
        elif alloc.kind == "ExternalOutput":
            out_avals.append(
                jax.core.ShapedArray(
                    tuple(alloc.tensor_shape), mybir.dt.np(alloc.dtype)
                )
            )
            out_names.append(name)
    n_params = len(in_names)
    all_in = tuple(in_names + out_names)
    if partition_name is not None:
        all_in = all_in + (partition_name,)

    def _body(*args):
        operands = list(args)
        if partition_name is not None:
            operands.append(bass2jax.partition_id_tensor())
        return tuple(
            bass2jax._bass_exec_p.bind(
                *operands,
                out_avals=tuple(out_avals),
                in_names=all_in,
                out_names=tuple(out_names),
                lowering_input_output_aliases=(),
                sim_require_finite=True,
                sim_require_nnan=True,
                nc=nc,
            )
        )

    devices = jax.devices()[:n_cores]
    mesh = Mesh(np.asarray(devices), ("core",))
    nspec = n_params + len(out_names)
    donate = tuple(range(n_params, nspec))
    sharded = jax.jit(
        shard_map(
            _body,
            mesh=mesh,
            in_specs=(PartitionSpec("core"),) * nspec,
            out_specs=(PartitionSpec("core"),) * len(out_names),
            check_rep=False,
        ),
        donate_argnums=donate,
        keep_unused=True,
    )
    sh = NamedSharding(mesh, PartitionSpec("core"))

    def run(in_maps):
        concat_in = [
            jax.device_put(
                np.concatenate([np.asarray(m[nm]) for m in in_maps], axis=0), sh
            )
            for nm in in_names
        ]
        import jax.numpy as jnp

        concat_zeros = [
            jax.device_put(
                jnp.zeros((n_cores * a.shape[0], *a.shape[1:]), a.dtype), sh
            )
            for a in out_avals
        ]
        jax.block_until_ready(concat_in)
        jax.block_until_ready(concat_zeros)
        out_arrs = sharded(*concat_in, *concat_zeros)
        jax.block_until_ready(out_arrs)
        return [
            {
                nm: np.asarray(out_arrs[i]).reshape(n_cores, *out_avals[i].shape)[c]
                for i, nm in enumerate(out_names)
            }
            for c in range(n_cores)
        ]

    return run


def _build():
    nc = bacc.Bacc("TRN2", target_bir_lowering=False, debug=False, num_devices=8)

    xq_d = nc.dram_tensor("xq", [P, 2, FR], BF16, kind="ExternalInput").ap()
    xk_d = nc.dram_tensor("xk", [P, NT, SLEN], BF16, kind="ExternalInput").ap()
    xv_d = nc.dram_tensor("xv", [P, NT, SLEN], BF16, kind="ExternalInput").ap()
    wpack_d = nc.dram_tensor("wpack", [P, WLEN], BF16, kind="ExternalInput").ap()
    wps_d = nc.dram_tensor("wps", [P, WSLEN], F32, kind="ExternalInput").ap()
    zz_d = nc.dram_tensor("zz", [64, 64], BF16, kind="ExternalInput").ap()

    out_d = nc.dram_tensor("out", [P, 2, FR], F32, kind="ExternalOutput").ap()

    gpart_d = nc.dram_tensor("g_part", [P, C], F32).ap()
    gred_d = nc.dram_tensor("g_red", [P, C], F32).ap()
    warm_d = nc.dram_tensor("cc_warm", [1, 64], F32).ap()
    warm_o = nc.dram_tensor("cc_warm_o", [1, 64], F32).ap()

    from concourse.tile_rust import add_dep_helper

    with tile.TileContext(nc) as tc:
        with (
            tc.tile_pool(name="const", bufs=1) as cpool,
            tc.tile_pool(name="img", bufs=2) as ipool,
            tc.tile_pool(name="xqp", bufs=1) as xqpool,
            tc.tile_pool(name="mid", bufs=4) as mpool,
            tc.tile_pool(name="kvt", bufs=2) as tpool,
            tc.tile_pool(name="work", bufs=1) as wpool,
            tc.tile_pool(name="ps", bufs=8, space="PSUM") as psp,
        ):
            # ---- phase-A constants: K-conv prerequisites only ----
            wp = cpool.tile([P, WLEN], BF16, tag="wp")
            wps_sb = cpool.tile([P, WSLEN], F32, tag="wps")
            nc.scalar.dma_start(
                wp[:, KW_O : KW_O + 2048], wpack_d[:, KW_O : KW_O + 2048]
            )
            nc.scalar.dma_start(wp[:, ID_O:], wpack_d[:, ID_O:])
            nc.scalar.dma_start(wps_sb[:], wps_d)
            qw_v = wp[:, QW_O : QW_O + 2048].rearrange(
                "p (i d o) -> p i d o", i=2, d=4
            )
            kw_v = wp[:, KW_O : KW_O + 2048].rearrange(
                "p (i d o) -> p i d o", i=2, d=4
            )
            vw_v = wp[:, VW_O : VW_O + 2048].rearrange(
                "p (i d o) -> p i d o", i=2, d=4
            )
            fw1_v = wp[:, FW1_O : FW1_O + 512].rearrange("p (i o) -> p i o", i=2)
            fw2_v = wp[:, FW2_O : FW2_O + 512].rearrange("p (i o) -> p i o", i=2)
            id_v = wp[:, ID_O : ID_O + P]

            def sca(off):  # [P, 1] fp32 per-partition scalar view
                return wps_sb[:, off : off + 2]

            xk_sb = ipool.tile([P, NT, SLEN], BF16, tag="big")
            nc.gpsimd.collective_compute(
                "AllReduce",
                ADD,
                replica_groups=[[0, 1], [2, 3], [4, 5], [6, 7]],
                ins=[warm_d],
                outs=[warm_o],
            )
            xk_dmas = [
                nc.gpsimd.dma_start(xk_sb[:, s], xk_d[:, s])
                for s in range(NT)
            ]

            flip = [0]

            def cast_copy(dst, src):
                if flip[0] % 2:
                    nc.scalar.copy(dst, src)
                else:
                    nc.vector.tensor_copy(dst, src)
                flip[0] += 1

            def conv_strip(src_sb, w_v, bias_off, s, nm):
                cs = mpool.tile([P, 2, TW], BF16, tag="mid", name=f"cs_{nm}{s}")
                sv2 = src_sb[:, s].rearrange("p (i f) -> p i f", i=2)
                for occ in range(2):
                    ps = psp.tile([P, TW], F32, tag="ps")
                    psv = ps[:].rearrange("p (i j) -> p i j", j=NJ)
                    first = True
                    for icc in range(2):
                        sv = sv2[:, icc, :].rearrange("p (r w) -> p r w", w=W_IMG)
                        for dd in range(4):
                            di, dj = dd // 2, dd % 2
                            nc.tensor.matmul(
                                psv,
                                w_v[:, icc, dd, occ * P : (occ + 1) * P],
                                sv[:, di::2, dj::2],
                                start=first,
                                stop=(icc == 1 and dd == 3),
                            )
                            first = False
                    dsl = cs[:, occ, :]
                    if flip[0] % 2:
                        nc.scalar.activation(
                            dsl, ps[:], IDENT,
                            bias=sca(bias_off)[:, occ : occ + 1], scale=1.0,
                        )
                    else:
                        nc.vector.tensor_tensor(
                            dsl, ps[:],
                            sca(bias_off)[:, occ : occ + 1].to_broadcast([P, TW]),
                            ADD,
                        )
                    flip[0] += 1
                return cs

            # ---- K conv + transposes (kt_all holds all chunks) ----
            kt_all = wpool.tile([P, NMT, C], BF16, tag="ktall")
            for s in range(NT):
                ks = conv_strip(xk_sb, kw_v, KB_O, s, "k")
                for c in range(3):
                    t = 3 * s + c
                    for occ in range(2):
                        tp = psp.tile([P, P], BF16, tag="ps")
                        nc.tensor.transpose(
                            tp[:], ks[:, occ, c * P : (c + 1) * P], id_v
                        )
                        cast_copy(kt_all[:, t, occ * P : (occ + 1) * P], tp[:])

            # ---- phase-B loads (serialized behind the K strips) ----
            xv_sb = ipool.tile([P, NT, SLEN], BF16, tag="big")
            d = nc.sync.dma_start(
                wp[:, VW_O : VW_O + 2048], wpack_d[:, VW_O : VW_O + 2048]
            )
            add_dep_helper(d.ins, xk_dmas[-1].ins, reason="phase loads")
            xv_dmas = []
            for s in range(NT):
                d = nc.sync.dma_start(xv_sb[:, s], xv_d[:, s])
                add_dep_helper(d.ins, xk_dmas[-1].ins, reason="phase loads")
                xv_dmas.append(d)

            # ---- V conv + transposes + streamed Gram accumulation ----
            gps = [
                psp.tile([P, C], F32, tag="ps", name=f"gps{g}") for g in range(2)
            ]
            for s in range(NT):
                vs = conv_strip(xv_sb, vw_v, VB_O, s, "v")
                for c in range(3):
                    t = 3 * s + c
                    vtt = tpool.tile([P, C], BF16, tag="vtt")
                    for occ in range(2):
                        tp = psp.tile([P, P], BF16, tag="ps")
                        nc.tensor.transpose(
                            tp[:], vs[:, occ, c * P : (c + 1) * P], id_v
                        )
                        cast_copy(vtt[:, occ * P : (occ + 1) * P], tp[:])
                    for g in range(2):
                        gmm = nc.tensor.matmul(
                            gps[g][:],
                            vtt[:, g * P : (g + 1) * P],
                            kt_all[:, t, :],
                            start=(t == 0),
                            stop=(t == NMT - 1),
                            skip_group_check=True,
                        )

            # ---- Gram exchange across the core pair ----
            gstage = wpool.tile([P, 2, P], F32, tag="gstage")
            for g in range(2):
                nc.vector.tensor_copy(
                    gstage[:, g, :], gps[g][:, g * P : (g + 1) * P]
                )
            nc.sync.dma_start(gpart_d, gstage[:])
            nc.gpsimd.collective_compute(
                "AllReduce",
                ADD,
                replica_groups=[[0, 1], [2, 3], [4, 5], [6, 7]],
                ins=[gpart_d],
                outs=[gred_d],
            )
            g_sb = wpool.tile([P, 2, P], BF16, tag="gsb")
            for g in range(2):
                for hh in range(2):
                    r0 = 64 * hh
                    r1 = 64 - r0
                    nc.gpsimd.dma_start(
                        g_sb[r0 : r0 + 64, g, r0 : r0 + 64],
                        gred_d[r0 : r0 + 64, g * P + r0 : g * P + r0 + 64],
                    )
                    nc.sync.dma_start(
                        g_sb[r0 : r0 + 64, g, r1 : r1 + 64], zz_d
                    )

            # ---- phase-C loads (Q conv + pose prerequisites) ----
            d = nc.sync.dma_start(
                wp[:, QW_O : QW_O + 2048], wpack_d[:, QW_O : QW_O + 2048]
            )
            add_dep_helper(d.ins, xv_dmas[-1].ins, reason="phase loads")
            xq_sb = xqpool.tile([P, 2, FR], BF16, tag="xq")
            d = nc.sync.dma_start(xq_sb[:], xq_d)
            add_dep_helper(d.ins, xv_dmas[-1].ins, reason="phase loads")
            d = nc.sync.dma_start(
                wp[:, FW1_O : FW1_O + 1024], wpack_d[:, FW1_O : FW1_O + 1024]
            )
            add_dep_helper(d.ins, xv_dmas[-1].ins, reason="phase loads")

            # ---- Q conv (fills the collective latency window) ----
            q_sb = wpool.tile([P, 2, NLOC], BF16, tag="q")
            xqv = [
                xq_sb[:, icc, :].rearrange("p (r w) -> p r w", w=W_IMG)
                for icc in range(2)
            ]
            for qcc in range(2):
                for nt in range(NT):
                    ps = psp.tile([P, TW], F32, tag="ps")
                    psv = ps[:].rearrange("p (i j) -> p i j", j=NJ)
                    first = True
                    for icc in range(2):
                        for dd in range(4):
                            di, dj = dd // 2, dd % 2
                            mm = nc.tensor.matmul(
                                psv,
                                qw_v[:, icc, dd, qcc * P : (qcc + 1) * P],
                                xqv[icc][:, 16 * nt + di : 16 * nt + 16 : 2, dj::2],
                                start=first,
                                stop=(icc == 1 and dd == 3),
                            )
                            if first:
                                add_dep_helper(
                                    mm.ins, gmm.ins, sync=False,
                                    reason="pin Q conv after Gram",
                                )
                            first = False
                    if nt % 2:
                        nc.scalar.activation(
                            q_sb[:, qcc, nt * TW : (nt + 1) * TW], ps[:], IDENT,
                            bias=sca(QB_O)[:, qcc : qcc + 1], scale=1.0,
                        )
                    else:
                        nc.vector.tensor_tensor(
                            q_sb[:, qcc, nt * TW : (nt + 1) * TW], ps[:],
                            sca(QB_O)[:, qcc : qcc + 1].to_broadcast([P, TW]),
                            ADD,
                        )

            # ---- pose term: matmul + copy to staging (no z dependency) ----
            stages = []
            xqr = [
                xq_sb[:, icc, :].rearrange("p (r w) -> p r w", w=W_IMG)
                for icc in range(2)
            ]
            for oc in range(2):
                ost = ipool.tile([P, FR], F32, tag="big", name=f"ost{oc}")
                stages.append(ost)
                for ot in range(OT):
                    h, k = ot // 6, ot % 6
                    for ri in range(2):
                        ps = psp.tile([P, 192], F32, tag="ps")
                        psv = ps[:].rearrange(
                            "p (rj i j) -> p rj i j", rj=2, i=2
                        )
                        for icc in range(2):
                            nc.tensor.matmul(
                                psv,
                                fw1_v[:, icc, oc * P : (oc + 1) * P],
                                xqr[icc][
                                    :, 4 * ot + ri : 4 * ot + ri + 3 : 2, :
                                ].rearrange("p i (j rj) -> p rj i j", rj=2),
                                start=(icc == 0),
                                stop=(icc == 1),
                            )
                        dst = ost[
                            :, h * 2304 + ri * 1152 : h * 2304 + ri * 1152 + 1152
                        ].rearrange("p (rj f) -> p rj f", rj=2)[
                            :, :, 96 * k : 96 * k + 96
                        ]
                        src_v = ps[:].rearrange("p (rj f) -> p rj f", rj=2)
                        if (ot + ri) % 2:
                            nc.scalar.copy(dst, src_v)
                        else:
                            nc.vector.tensor_copy(dst, src_v)

            # ---- W_cat^T = blockdiag(G) @ fw2'^T  (gamma/8 pre-folded) ----
            w_sb = wpool.tile([P, 2, C], BF16, tag="w")
            for g in range(2):
                psw = psp.tile([P, C], F32, tag="ps")
                nc.tensor.matmul(
                    psw[:], g_sb[:, g, :], fw2_v[:, g, :], start=True, stop=True
                )
                nc.scalar.copy(w_sb[:, g, :], psw[:])

            # ---- z'' = W_cat^T.T @ Q + fb ----
            z_sb = wpool.tile([P, 2, NLOC], F32, tag="z")
            for oc in range(2):
                for nt in range(NT):
                    ps = psp.tile([P, TW], F32, tag="ps")
                    for g in range(2):
                        nc.tensor.matmul(
                            ps[:],
                            w_sb[:, g, oc * P : (oc + 1) * P],
                            q_sb[:, g, nt * TW : (nt + 1) * TW],
                            start=(g == 0),
                            stop=(g == 1),
                        )
                    nc.scalar.activation(
                        z_sb[:, oc, nt * TW : (nt + 1) * TW], ps[:], IDENT,
                        bias=sca(FB_O)[:, oc : oc + 1], scale=1.0,
                    )

            # ---- late pass: accumulate up2x(z'') into the staging via the
            #      DMA's inline adder (parity layout makes it plain slabs) ----
            for oc in range(2):
                ost = stages[oc]
                for half in range(2):
                    zslab = z_sb[:, oc, 576 * half : 576 * half + 576]
                    for rr in range(4):
                        off = half * 2304 + rr * 576
                        nc.gpsimd.dma_start(
                            ost[:, off : off + 576], zslab, accum_op=ADD
                        )
                    nc.sync.dma_start(
                        out_d[:, oc, half * 2304 : half * 2304 + 2304],
                        ost[:, half * 2304 : half * 2304 + 2304],
                    )

    nc.compile()
    return nc


def _prep_inputs(inputs):
    """Build the 8 per-core input maps (host-side shard + weight packing)."""
    import ml_dtypes

    f = np.float32
    b16 = ml_dtypes.bfloat16
    qw, qb = np.asarray(inputs["qw"], f), np.asarray(inputs["qb"], f)
    kw, kb = np.asarray(inputs["kw"], f), np.asarray(inputs["kb"], f)
    vw, vb = np.asarray(inputs["vw"], f), np.asarray(inputs["vb"], f)
    gamma = np.asarray(inputs["gamma"], f)
    fw, fb = np.asarray(inputs["fw"], f), np.asarray(inputs["fb"], f)
    pose = np.asarray(inputs["pose_enc"], f)
    app_pose = np.asarray(inputs["app_pose_enc"], f)
    app = np.asarray(inputs["app_enc"], f)

    wpack = np.zeros((P, WLEN), dtype=b16)
    wps = np.zeros((P, WSLEN), dtype=f)

    def packw(dst_off, w):
        # w [oc, ic, 2, 2] -> [p, icc, dd, oc]
        t = w.transpose(1, 2, 3, 0).reshape(2, P, 4, C).transpose(1, 0, 2, 3)
        wpack[:, dst_off : dst_off + 2048] = t.reshape(P, 2048).astype(b16)

    packw(QW_O, qw)
    packw(KW_O, kw)
    packw(VW_O, vw)
    wpack[:, FW1_O : FW1_O + 512] = (
        fw[:, :C, 0, 0].T.reshape(2, P, C).transpose(1, 0, 2).reshape(P, 512)
    ).astype(b16)
    gsc = (np.repeat(gamma.astype(np.float64), 64) / 8.0)[:, None]
    fw2s = (fw[:, C:, 0, 0].T.astype(np.float64) * gsc).astype(f)
    wpack[:, FW2_O : FW2_O + 512] = (
        fw2s.reshape(2, P, C).transpose(1, 0, 2).reshape(P, 512)
    ).astype(b16)
    wpack[:, ID_O : ID_O + P] = np.eye(P, dtype=f).astype(b16)
    wps[:, QB_O : QB_O + 2] = qb.reshape(2, P).T
    wps[:, FB_O : FB_O + 2] = fb.reshape(2, P).T
    wps[:, KB_O : KB_O + 2] = kb.reshape(2, P).T
    wps[:, VB_O : VB_O + 2] = vb.reshape(2, P).T
    zz = np.zeros((64, 64), dtype=b16)

    def shard_q(x, b, h):  # [p, icc, fr]
        halfimg = x[b, :, RH * h : RH * (h + 1), :].reshape(2, P, FR)
        return halfimg.transpose(1, 0, 2).astype(b16)

    def shard_kv(x, b, h):  # [p, strip, icc*1536]
        halfimg = x[b, :, RH * h : RH * (h + 1), :].reshape(2, P, NT, SLEN // 2)
        return halfimg.transpose(1, 2, 0, 3).reshape(P, NT, SLEN).astype(b16)

    in_maps = []
    for c in range(8):
        b, h = c // 2, c % 2
        in_maps.append({
            "xq": shard_q(pose, b, h),
            "xk": shard_kv(app_pose, b, h),
            "xv": shard_kv(app, b, h),
            "wpack": wpack,
            "wps": wps,
            "zz": zz,
        })
    return in_maps


def _get_runner():
    global _CACHED_NC, _RUNNER
    if _CACHED_NC is None:
        _CACHED_NC = _build()
    if _RUNNER is None:
        _RUNNER = _make_runner(_CACHED_NC)
    return _RUNNER


def _assemble(results):
    out = np.empty((4, C, W_IMG, W_IMG), dtype=np.float32)
    for c in range(8):
        b, h = c // 2, c % 2
        o = results[c]["out"]  # parity layout [P, oc, half, ri, rj, i, j]
        o = o.reshape(P, 2, 2, 2, 2, 12, NJ)
        o = o.transpose(1, 0, 2, 5, 3, 6, 4)  # [oc, p, half, i, ri, j, rj]
        out[b, :, RH * h : RH * (h + 1), :] = o.reshape(C, RH, W_IMG)
    return out


def kernel(**inputs):
    run = _get_runner()
    in_maps = _prep_inputs(inputs)
    return _assemble(run(in_maps))


# revision 38
# speedup vs baseline: 1.2554x; 1.2554x over previous
"""Trainium2 Bass kernel for nn_AttnMech (sparse_attention, no-softmax attention).

Math (reference):
  q/k/v = 2x2-stride-2 convs of pose/app_pose/app  -> [B, 4*64, 48, 48]
  attn  = (Q^T K)/8 (no softmax);  out = attn @ V^T
  out   = gamma_h * out; nearest-upsample 2x; concat with pose; 1x1 conv.

Key algebraic restructure (linear attention => reassociate):
  out_h = V_h (Q_h^T K_h)^T / 8 = (V_h K_h^T) Q_h / 8 = G_h Q_h / 8
with G_h = V_h K_h^T a tiny 64x64 Gram matrix.  The per-head projection,
upsample and final 1x1 conv then fold into:
  final = fw1 @ pose_enc + up2x( W_cat @ Q + fb ) ,
  W_cat[:, 64h:64h+64] = (gamma_h/8) * fw2_h @ G_h
so the huge [2304,2304] attention matrices never exist.

Sharding over 8 cores: core c = (image b = c//2, spatial half = c%2).
Each core convs its half of the image; partial Gram matrices are
AllReduced across the core pair (64 KB); everything else is local.
All matmuls run as float32r (full PE rate for free-dim >= 256).

Implementation notes:
  - All convs keep weights as the stationary operand (single free dim as
    the hardware requires) and stream the image with multi-dim strided
    moving APs.  K/V results are then PE-transposed per 128-pixel chunk
    and immediately consumed by the Gram accumulation.
  - All constants ship in one packed [128, WLEN] DRAM blob (one DMA,
    one descriptor run per partition); image tensors are laid out
    host-side so every DMA is one contiguous run per partition.
  - The Q conv is scheduled after the AllReduce launch so the PE has
    work while the collective's ~20us fixed latency elapses.
"""

import os
import sys

for _p in ("/opt/trn_rl_repo", "/root/.axon_site/_ro/trn_rl_repo"):
    if os.path.isdir(_p) and _p not in sys.path:
        sys.path.insert(0, _p)

import numpy as np

import concourse.mybir as mybir
import concourse.tile as tile
from concourse import bacc, bass2jax

F32 = mybir.dt.float32
F32R = mybir.dt.float32r
BF16 = mybir.dt.bfloat16
ADD = mybir.AluOpType.add
IDENT = mybir.ActivationFunctionType.Identity

P = 128          # partitions
C = 256          # channels
W_IMG = 96       # full-res width
RH = 48          # rows per half (full-res)
FR = RH * W_IMG  # 4608 flat half-image
NI = 24          # local downsampled rows
NJ = 48          # downsampled cols
NLOC = NI * NJ   # 1152 local attn pixels
NT = 3           # conv free tiles of 384 (16 full-res rows each)
TW = 384
SLEN = 2 * 16 * W_IMG  # strip len per partition (both ic chunks) 3072
NMT = 9          # 128-pixel chunks of the local grid
OT = 12          # output assembly tiles of 384 (4 full-res rows)

# wpack layout (per partition, bf16 words) + separate fp32 bias blob
QW_O, KW_O, VW_O = 0, 2048, 4096
FW1_O, FW2_O = 6144, 6656
ID_O = 7168
WLEN = 7296
QB_O, FB_O, KB_O, VB_O = 0, 2, 4, 6
WSLEN = 8

_CACHED_NC = None
_RUNNER = None


def _make_runner(nc, n_cores=8):
    """Like bass2jax.run_bass_via_pjrt, but inputs are pre-placed on the
    devices (parallel transfer + aligned core start) and the jitted
    executable is cached across calls."""
    import jax
    from jax.experimental.shard_map import shard_map
    from jax.sharding import Mesh, NamedSharding, PartitionSpec

    bass2jax.install_neuronx_cc_hook()

    partition_name = (
        nc.partition_id_tensor.name if nc.partition_id_tensor else None
    )
    in_names, out_names, out_avals = [], [], []
    for alloc in nc.m.functions[0].allocations:
        if not isinstance(alloc, mybir.MemoryLocationSet):
            continue
        name = alloc.memorylocations[0].name
        if alloc.kind == "ExternalInput":
            if name != partition_name:
                in_names.append(name)
        elif alloc.kind == "ExternalOutput":
            out_avals.append(
                jax.core.ShapedArray(
                    tuple(alloc.tensor_shape), mybir.dt.np(alloc.dtype)
                )
            )
            out_names.append(name)
    n_params = len(in_names)
    all_in = tuple(in_names + out_names)
    if partition_name is not None:
        all_in = all_in + (partition_name,)

    def _body(*args):
        operands = list(args)
        if partition_name is not None:
            operands.append(bass2jax.partition_id_tensor())
        return tuple(
            bass2jax._bass_exec_p.bind(
                *operands,
                out_avals=tuple(out_avals),
                in_names=all_in,
                out_names=tuple(out_names),
                lowering_input_output_aliases=(),
                sim_require_finite=True,
                sim_require_nnan=True,
                nc=nc,
            )
        )

    devices = jax.devices()[:n_cores]
    mesh = Mesh(np.asarray(devices), ("core",))
    nspec = n_params + len(out_names)
    donate = tuple(range(n_params, nspec))
    sharded = jax.jit(
        shard_map(
            _body,
            mesh=mesh,
            in_specs=(PartitionSpec("core"),) * nspec,
            out_specs=(PartitionSpec("core"),) * len(out_names),
            check_rep=False,
        ),
        donate_argnums=donate,
        keep_unused=True,
    )
    sh = NamedSharding(mesh, PartitionSpec("core"))

    def run(in_maps):
        concat_in = [
            jax.device_put(
                np.concatenate([np.asarray(m[nm]) for m in in_maps], axis=0), sh
            )
            for nm in in_names
        ]
        import jax.numpy as jnp

        concat_zeros = [
            jax.device_put(
                jnp.zeros((n_cores * a.shape[0], *a.shape[1:]), a.dtype), sh
            )
            for a in out_avals
        ]
        jax.block_until_ready(concat_in)
        jax.block_until_ready(concat_zeros)
        out_arrs = sharded(*concat_in, *concat_zeros)
        jax.block_until_ready(out_arrs)
        return [
            {
                nm: np.asarray(out_arrs[i]).reshape(n_cores, *out_avals[i].shape)[c]
                for i, nm in enumerate(out_names)
            }
            for c in range(n_cores)
        ]

    return run


def _build():
    nc = bacc.Bacc("TRN2", target_bir_lowering=False, debug=False, num_devices=8)

    xq_d = nc.dram_tensor("xq", [P, 2, FR], BF16, kind="ExternalInput").ap()
    xk_d = nc.dram_tensor("xk", [P, NT, SLEN], BF16, kind="ExternalInput").ap()
    xv_d = nc.dram_tensor("xv", [P, NT, SLEN], BF16, kind="ExternalInput").ap()
    wpack_d = nc.dram_tensor("wpack", [P, WLEN], BF16, kind="ExternalInput").ap()
    wps_d = nc.dram_tensor("wps", [P, WSLEN], F32, kind="ExternalInput").ap()
    zz_d = nc.dram_tensor("zz", [64, 64], BF16, kind="ExternalInput").ap()

    out_d = nc.dram_tensor("out", [P, 2, FR], F32, kind="ExternalOutput").ap()

    gpart_d = nc.dram_tensor("g_part", [P, C], F32).ap()
    gred_d = nc.dram_tensor("g_red", [P, C], F32).ap()
    warm_d = nc.dram_tensor("cc_warm", [1, 64], F32).ap()
    warm_o = nc.dram_tensor("cc_warm_o", [1, 64], F32).ap()

    from concourse.tile_rust import add_dep_helper

    with tile.TileContext(nc) as tc:
        with (
            tc.tile_pool(name="const", bufs=1) as cpool,
            tc.tile_pool(name="img", bufs=2) as ipool,
            tc.tile_pool(name="xqp", bufs=1) as xqpool,
            tc.tile_pool(name="mid", bufs=4) as mpool,
            tc.tile_pool(name="kvt", bufs=2) as tpool,
            tc.tile_pool(name="work", bufs=1) as wpool,
            tc.tile_pool(name="ps", bufs=8, space="PSUM") as psp,
        ):
            # ---- phase-A constants: K-conv prerequisites only ----
            wp = cpool.tile([P, WLEN], BF16, tag="wp")
            wps_sb = cpool.tile([P, WSLEN], F32, tag="wps")
            nc.scalar.dma_start(
                wp[:, KW_O : KW_O + 2048], wpack_d[:, KW_O : KW_O + 2048]
            )
            nc.scalar.dma_start(wp[:, ID_O:], wpack_d[:, ID_O:])
            nc.scalar.dma_start(wps_sb[:], wps_d)
            qw_v = wp[:, QW_O : QW_O + 2048].rearrange(
                "p (i d o) -> p i d o", i=2, d=4
            )
            kw_v = wp[:, KW_O : KW_O + 2048].rearrange(
                "p (i d o) -> p i d o", i=2, d=4
            )
            vw_v = wp[:, VW_O : VW_O + 2048].rearrange(
                "p (i d o) -> p i d o", i=2, d=4
            )
            fw1_v = wp[:, FW1_O : FW1_O + 512].rearrange("p (i o) -> p i o", i=2)
            fw2_v = wp[:, FW2_O : FW2_O + 512].rearrange("p (i o) -> p i o", i=2)
            id_v = wp[:, ID_O : ID_O + P]

            def sca(off):  # [P, 1] fp32 per-partition scalar view
                return wps_sb[:, off : off + 2]

            xk_sb = ipool.tile([P, NT, SLEN], BF16, tag="big")
            nc.gpsimd.collective_compute(
                "AllReduce",
                ADD,
                replica_groups=[[0, 1], [2, 3], [4, 5], [6, 7]],
                ins=[warm_d],
                outs=[warm_o],
            )
            xk_dmas = [
                nc.gpsimd.dma_start(xk_sb[:, s], xk_d[:, s])
                for s in range(NT)
            ]

            flip = [0]

            def cast_copy(dst, src):
                if flip[0] % 2:
                    nc.scalar.copy(dst, src)
                else:
                    nc.vector.tensor_copy(dst, src)
                flip[0] += 1

            def conv_strip(src_sb, w_v, bias_off, s, nm):
                cs = mpool.tile([P, 2, TW], BF16, tag="mid", name=f"cs_{nm}{s}")
                sv2 = src_sb[:, s].rearrange("p (i f) -> p i f", i=2)
                for occ in range(2):
                    ps = psp.tile([P, TW], F32, tag="ps")
                    psv = ps[:].rearrange("p (i j) -> p i j", j=NJ)
                    first = True
                    for icc in range(2):
                        sv = sv2[:, icc, :].rearrange("p (r w) -> p r w", w=W_IMG)
                        for dd in range(4):
                            di, dj = dd // 2, dd % 2
                            nc.tensor.matmul(
                                psv,
                                w_v[:, icc, dd, occ * P : (occ + 1) * P],
                                sv[:, di::2, dj::2],
                                start=first,
                                stop=(icc == 1 and dd == 3),
                            )
                            first = False
                    dsl = cs[:, occ, :]
                    if flip[0] % 2:
                        nc.scalar.activation(
                            dsl, ps[:], IDENT,
                            bias=sca(bias_off)[:, occ : occ + 1], scale=1.0,
                        )
                    else:
                        nc.vector.tensor_tensor(
                            dsl, ps[:],
                            sca(bias_off)[:, occ : occ + 1].to_broadcast([P, TW]),
                            ADD,
                        )
                    flip[0] += 1
                return cs

            # ---- K conv + transposes (kt_all holds all chunks) ----
            kt_all = wpool.tile([P, NMT, C], BF16, tag="ktall")
            for s in range(NT):
                ks = conv_strip(xk_sb, kw_v, KB_O, s, "k")
                for c in range(3):
                    t = 3 * s + c
                    for occ in range(2):
                        tp = psp.tile([P, P], BF16, tag="ps")
                        nc.tensor.transpose(
                            tp[:], ks[:, occ, c * P : (c + 1) * P], id_v
                        )
                        cast_copy(kt_all[:, t, occ * P : (occ + 1) * P], tp[:])

            # ---- phase-B loads (serialized behind the K strips) ----
            xv_sb = ipool.tile([P, NT, SLEN], BF16, tag="big")
            d = nc.sync.dma_start(
                wp[:, VW_O : VW_O + 2048], wpack_d[:, VW_O : VW_O + 2048]
            )
            add_dep_helper(d.ins, xk_dmas[-1].ins, reason="phase loads")
            xv_dmas = []
            for s in range(NT):
                d = nc.sync.dma_start(xv_sb[:, s], xv_d[:, s])
                add_dep_helper(d.ins, xk_dmas[-1].ins, reason="phase loads")
                xv_dmas.append(d)

            # ---- V conv + transposes + streamed Gram accumulation ----
            gps = [
                psp.tile([P, C], F32, tag="ps", name=f"gps{g}") for g in range(2)
            ]
            for s in range(NT):
                vs = conv_strip(xv_sb, vw_v, VB_O, s, "v")
                for c in range(3):
                    t = 3 * s + c
                    vtt = tpool.tile([P, C], BF16, tag="vtt")
                    for occ in range(2):
                        tp = psp.tile([P, P], BF16, tag="ps")
                        nc.tensor.transpose(
                            tp[:], vs[:, occ, c * P : (c + 1) * P], id_v
                        )
                        cast_copy(vtt[:, occ * P : (occ + 1) * P], tp[:])
                    for g in range(2):
                        gmm = nc.tensor.matmul(
                            gps[g][:],
                            vtt[:, g * P : (g + 1) * P],
                            kt_all[:, t, :],
                            start=(t == 0),
                            stop=(t == NMT - 1),
                            skip_group_check=True,
                        )

            # ---- Gram exchange across the core pair ----
            gstage = wpool.tile([P, 2, P], F32, tag="gstage")
            for g in range(2):
                nc.vector.tensor_copy(
                    gstage[:, g, :], gps[g][:, g * P : (g + 1) * P]
                )
            nc.sync.dma_start(gpart_d, gstage[:])
            nc.gpsimd.collective_compute(
                "AllReduce",
                ADD,
                replica_groups=[[0, 1], [2, 3], [4, 5], [6, 7]],
                ins=[gpart_d],
                outs=[gred_d],
            )
            g_sb = wpool.tile([P, 2, P], BF16, tag="gsb")
            for g in range(2):
                for hh in range(2):
                    r0 = 64 * hh
                    r1 = 64 - r0
                    nc.gpsimd.dma_start(
                        g_sb[r0 : r0 + 64, g, r0 : r0 + 64],
                        gred_d[r0 : r0 + 64, g * P + r0 : g * P + r0 + 64],
                    )
                    nc.sync.dma_start(
                        g_sb[r0 : r0 + 64, g, r1 : r1 + 64], zz_d
                    )

            # ---- phase-C loads (Q conv + pose prerequisites) ----
            d = nc.sync.dma_start(
                wp[:, QW_O : QW_O + 2048], wpack_d[:, QW_O : QW_O + 2048]
            )
            add_dep_helper(d.ins, xv_dmas[-1].ins, reason="phase loads")
            xq_sb = xqpool.tile([P, 2, FR], BF16, tag="xq")
            d = nc.sync.dma_start(xq_sb[:], xq_d)
            add_dep_helper(d.ins, xv_dmas[-1].ins, reason="phase loads")
            d = nc.sync.dma_start(
                wp[:, FW1_O : FW1_O + 1024], wpack_d[:, FW1_O : FW1_O + 1024]
            )
            add_dep_helper(d.ins, xv_dmas[-1].ins, reason="phase loads")

            # ---- Q conv (fills the collective latency window) ----
            q_sb = wpool.tile([P, 2, NLOC], BF16, tag="q")
            xqv = [
                xq_sb[:, icc, :].rearrange("p (r w) -> p r w", w=W_IMG)
                for icc in range(2)
            ]
            for qcc in range(2):
                for nt in range(NT):
                    ps = psp.tile([P, TW], F32, tag="ps")
                    psv = ps[:].rearrange("p (i j) -> p i j", j=NJ)
                    first = True
                    for icc in range(2):
                        for dd in range(4):
                            di, dj = dd // 2, dd % 2
                            mm = nc.tensor.matmul(
                                psv,
                                qw_v[:, icc, dd, qcc * P : (qcc + 1) * P],
                                xqv[icc][:, 16 * nt + di : 16 * nt + 16 : 2, dj::2],
                                start=first,
                                stop=(icc == 1 and dd == 3),
                            )
                            if first:
                                add_dep_helper(
                                    mm.ins, gmm.ins, sync=False,
                                    reason="pin Q conv after Gram",
                                )
                            first = False
                    if nt % 2:
                        nc.scalar.activation(
                            q_sb[:, qcc, nt * TW : (nt + 1) * TW], ps[:], IDENT,
                            bias=sca(QB_O)[:, qcc : qcc + 1], scale=1.0,
                        )
                    else:
                        nc.vector.tensor_tensor(
                            q_sb[:, qcc, nt * TW : (nt + 1) * TW], ps[:],
                            sca(QB_O)[:, qcc : qcc + 1].to_broadcast([P, TW]),
                            ADD,
                        )

            # ---- pose term: matmul + copy to staging (no z dependency) ----
            stages = []
            for oc in range(2):
                ost = ipool.tile([P, FR], BF16, tag="big", name=f"ost{oc}")
                stages.append(ost)
                for ot in range(OT):
                    ps = psp.tile([P, TW], F32, tag="ps")
                    for icc in range(2):
                        nc.tensor.matmul(
                            ps[:],
                            fw1_v[:, icc, oc * P : (oc + 1) * P],
                            xq_sb[:, icc, ot * TW : (ot + 1) * TW],
                            start=(icc == 0),
                            stop=(icc == 1),
                        )
                    if ot % 2:
                        nc.scalar.copy(ost[:, ot * TW : (ot + 1) * TW], ps[:])
                    else:
                        nc.vector.tensor_copy(
                            ost[:, ot * TW : (ot + 1) * TW], ps[:]
                        )

            # ---- W_cat^T = blockdiag(G) @ fw2'^T  (gamma/8 pre-folded) ----
            w_sb = wpool.tile([P, 2, C], BF16, tag="w")
            for g in range(2):
                psw = psp.tile([P, C], F32, tag="ps")
                nc.tensor.matmul(
                    psw[:], g_sb[:, g, :], fw2_v[:, g, :], start=True, stop=True
                )
                nc.scalar.copy(w_sb[:, g, :], psw[:])

            # ---- z'' = W_cat^T.T @ Q + fb ----
            z_sb = wpool.tile([P, 2, NLOC], F32, tag="z")
            for oc in range(2):
                for nt in range(NT):
                    ps = psp.tile([P, TW], F32, tag="ps")
                    for g in range(2):
                        nc.tensor.matmul(
                            ps[:],
                            w_sb[:, g, oc * P : (oc + 1) * P],
                            q_sb[:, g, nt * TW : (nt + 1) * TW],
                            start=(g == 0),
                            stop=(g == 1),
                        )
                    nc.scalar.activation(
                        z_sb[:, oc, nt * TW : (nt + 1) * TW], ps[:], IDENT,
                        bias=sca(FB_O)[:, oc : oc + 1], scale=1.0,
                    )

            # ---- late pass: obuf = staging + up2x(z''), then store ----
            for oc in range(2):
                ost = stages[oc]
                zv = z_sb[:, oc, :].rearrange("p (i j) -> p i j", j=NJ)
                for half in range(2):
                    obuf = mpool.tile([P, 6 * TW], F32, tag="obuf")
                    stv = ost[:, half * 6 * TW : (half + 1) * 6 * TW].rearrange(
                        "p (i ri j rj) -> p i ri j rj", i=12, ri=2, j=NJ, rj=2
                    )
                    ov = obuf[:].rearrange(
                        "p (i ri j rj) -> p i ri j rj", i=12, ri=2, j=NJ, rj=2
                    )
                    zb = zv[:, 12 * half : 12 * (half + 1), :, None].to_broadcast(
                        [P, 12, NJ, 2]
                    )
                    for ri in range(2):
                        idx = oc * 4 + half * 2 + ri
                        eng = nc.gpsimd if idx in (0, 3, 6) else nc.vector
                        eng.tensor_tensor(
                            ov[:, :, ri, :, :], stv[:, :, ri, :, :], zb, ADD
                        )
                    nc.sync.dma_start(
                        out_d[:, oc, half * 6 * TW : (half + 1) * 6 * TW],
                        obuf[:],
                    )

    nc.compile()
    return nc


def _prep_inputs(inputs):
    """Build the 8 per-core input maps (host-side shard + weight packing)."""
    import ml_dtypes

    f = np.float32
    b16 = ml_dtypes.bfloat16
    qw, qb = np.asarray(inputs["qw"], f), np.asarray(inputs["qb"], f)
    kw, kb = np.asarray(inputs["kw"], f), np.asarray(inputs["kb"], f)
    vw, vb = np.asarray(inputs["vw"], f), np.asarray(inputs["vb"], f)
    gamma = np.asarray(inputs["gamma"], f)
    fw, fb = np.asarray(inputs["fw"], f), np.asarray(inputs["fb"], f)
    pose = np.asarray(inputs["pose_enc"], f)
    app_pose = np.asarray(inputs["app_pose_enc"], f)
    app = np.asarray(inputs["app_enc"], f)

    wpack = np.zeros((P, WLEN), dtype=b16)
    wps = np.zeros((P, WSLEN), dtype=f)

    def packw(dst_off, w):
        # w [oc, ic, 2, 2] -> [p, icc, dd, oc]
        t = w.transpose(1, 2, 3, 0).reshape(2, P, 4, C).transpose(1, 0, 2, 3)
        wpack[:, dst_off : dst_off + 2048] = t.reshape(P, 2048).astype(b16)

    packw(QW_O, qw)
    packw(KW_O, kw)
    packw(VW_O, vw)
    wpack[:, FW1_O : FW1_O + 512] = (
        fw[:, :C, 0, 0].T.reshape(2, P, C).transpose(1, 0, 2).reshape(P, 512)
    ).astype(b16)
    gsc = (np.repeat(gamma.astype(np.float64), 64) / 8.0)[:, None]
    fw2s = (fw[:, C:, 0, 0].T.astype(np.float64) * gsc).astype(f)
    wpack[:, FW2_O : FW2_O + 512] = (
        fw2s.reshape(2, P, C).transpose(1, 0, 2).reshape(P, 512)
    ).astype(b16)
    wpack[:, ID_O : ID_O + P] = np.eye(P, dtype=f).astype(b16)
    wps[:, QB_O : QB_O + 2] = qb.reshape(2, P).T
    wps[:, FB_O : FB_O + 2] = fb.reshape(2, P).T
    wps[:, KB_O : KB_O + 2] = kb.reshape(2, P).T
    wps[:, VB_O : VB_O + 2] = vb.reshape(2, P).T
    zz = np.zeros((64, 64), dtype=b16)

    def shard_q(x, b, h):  # [p, icc, fr]
        halfimg = x[b, :, RH * h : RH * (h + 1), :].reshape(2, P, FR)
        return halfimg.transpose(1, 0, 2).astype(b16)

    def shard_kv(x, b, h):  # [p, strip, icc*1536]
        halfimg = x[b, :, RH * h : RH * (h + 1), :].reshape(2, P, NT, SLEN // 2)
        return halfimg.transpose(1, 2, 0, 3).reshape(P, NT, SLEN).astype(b16)

    in_maps = []
    for c in range(8):
        b, h = c // 2, c % 2
        in_maps.append({
            "xq": shard_q(pose, b, h),
            "xk": shard_kv(app_pose, b, h),
            "xv": shard_kv(app, b, h),
            "wpack": wpack,
            "wps": wps,
            "zz": zz,
        })
    return in_maps


def _get_runner():
    global _CACHED_NC, _RUNNER
    if _CACHED_NC is None:
        _CACHED_NC = _build()
    if _RUNNER is None:
        _RUNNER = _make_runner(_CACHED_NC)
    return _RUNNER


def _assemble(results):
    out = np.empty((4, C, W_IMG, W_IMG), dtype=np.float32)
    for c in range(8):
        b, h = c // 2, c % 2
        o = results[c]["out"]  # [P, 2, FR]
        out[b, :, RH * h : RH * (h + 1), :] = o.transpose(1, 0, 2).reshape(
            C, RH, W_IMG
        )
    return out


def kernel(**inputs):
    run = _get_runner()
    in_maps = _prep_inputs(inputs)
    return _assemble(run(in_maps))


# revision 39
# speedup vs baseline: 1.3188x; 1.0505x over previous
"""Trainium2 Bass kernel for nn_AttnMech (sparse_attention, no-softmax attention).

Math (reference):
  q/k/v = 2x2-stride-2 convs of pose/app_pose/app  -> [B, 4*64, 48, 48]
  attn  = (Q^T K)/8 (no softmax);  out = attn @ V^T
  out   = gamma_h * out; nearest-upsample 2x; concat with pose; 1x1 conv.

Key algebraic restructure (linear attention => reassociate):
  out_h = V_h (Q_h^T K_h)^T / 8 = (V_h K_h^T) Q_h / 8 = G_h Q_h / 8
with G_h = V_h K_h^T a tiny 64x64 Gram matrix.  The per-head projection,
upsample and final 1x1 conv then fold into:
  final = fw1 @ pose_enc + up2x( W_cat @ Q + fb ) ,
  W_cat[:, 64h:64h+64] = (gamma_h/8) * fw2_h @ G_h
so the huge [2304,2304] attention matrices never exist.

Sharding over 8 cores: core c = (image b = c//2, spatial half = c%2).
Each core convs its half of the image; partial Gram matrices are
AllReduced across the core pair (64 KB); everything else is local.
All matmuls run as float32r (full PE rate for free-dim >= 256).

Implementation notes:
  - All convs keep weights as the stationary operand (single free dim as
    the hardware requires) and stream the image with multi-dim strided
    moving APs.  K/V results are then PE-transposed per 128-pixel chunk
    and immediately consumed by the Gram accumulation.
  - All constants ship in one packed [128, WLEN] DRAM blob (one DMA,
    one descriptor run per partition); image tensors are laid out
    host-side so every DMA is one contiguous run per partition.
  - The Q conv is scheduled after the AllReduce launch so the PE has
    work while the collective's ~20us fixed latency elapses.
"""

import os
import sys

for _p in ("/opt/trn_rl_repo", "/root/.axon_site/_ro/trn_rl_repo"):
    if os.path.isdir(_p) and _p not in sys.path:
        sys.path.insert(0, _p)

import numpy as np

import concourse.mybir as mybir
import concourse.tile as tile
from concourse import bacc, bass2jax

F32 = mybir.dt.float32
F32R = mybir.dt.float32r
BF16 = mybir.dt.bfloat16
ADD = mybir.AluOpType.add
IDENT = mybir.ActivationFunctionType.Identity

P = 128          # partitions
C = 256          # channels
W_IMG = 96       # full-res width
RH = 48          # rows per half (full-res)
FR = RH * W_IMG  # 4608 flat half-image
NI = 24          # local downsampled rows
NJ = 48          # downsampled cols
NLOC = NI * NJ   # 1152 local attn pixels
NT = 3           # conv free tiles of 384 (16 full-res rows each)
TW = 384
SLEN = 2 * 16 * W_IMG  # strip len per partition (both ic chunks) 3072
NMT = 9          # 128-pixel chunks of the local grid
OT = 12          # output assembly tiles of 384 (4 full-res rows)

# wpack layout (per partition, bf16 words) + separate fp32 bias blob
QW_O, KW_O, VW_O = 0, 2048, 4096
FW1_O, FW2_O = 6144, 6656
ID_O = 7168
WLEN = 7296
QB_O, FB_O, KB_O, VB_O = 0, 2, 4, 6
WSLEN = 8

_CACHED_NC = None
_RUNNER = None


def _make_runner(nc, n_cores=8):
    """Like bass2jax.run_bass_via_pjrt, but inputs are pre-placed on the
    devices (parallel transfer + aligned core start) and the jitted
    executable is cached across calls."""
    import jax
    from jax.experimental.shard_map import shard_map
    from jax.sharding import Mesh, NamedSharding, PartitionSpec

    bass2jax.install_neuronx_cc_hook()

    partition_name = (
        nc.partition_id_tensor.name if nc.partition_id_tensor else None
    )
    in_names, out_names, out_avals = [], [], []
    for alloc in nc.m.functions[0].allocations:
        if not isinstance(alloc, mybir.MemoryLocationSet):
            continue
        name = alloc.memorylocations[0].name
        if alloc.kind == "ExternalInput":
            if name != partition_name:
                in_names.append(name)
        elif alloc.kind == "ExternalOutput":
            out_avals.append(
                jax.core.ShapedArray(
                    tuple(alloc.tensor_shape), mybir.dt.np(alloc.dtype)
                )
            )
            out_names.append(name)
    n_params = len(in_names)
    all_in = tuple(in_names + out_names)
    if partition_name is not None:
        all_in = all_in + (partition_name,)

    def _body(*args):
        operands = list(args)
        if partition_name is not None:
            operands.append(bass2jax.partition_id_tensor())
        return tuple(
            bass2jax._bass_exec_p.bind(
                *operands,
                out_avals=tuple(out_avals),
                in_names=all_in,
                out_names=tuple(out_names),
                lowering_input_output_aliases=(),
                sim_require_finite=True,
                sim_require_nnan=True,
                nc=nc,
            )
        )

    devices = jax.devices()[:n_cores]
    mesh = Mesh(np.asarray(devices), ("core",))
    nspec = n_params + len(out_names)
    donate = tuple(range(n_params, nspec))
    sharded = jax.jit(
        shard_map(
            _body,
            mesh=mesh,
            in_specs=(PartitionSpec("core"),) * nspec,
            out_specs=(PartitionSpec("core"),) * len(out_names),
            check_rep=False,
        ),
        donate_argnums=donate,
        keep_unused=True,
    )
    sh = NamedSharding(mesh, PartitionSpec("core"))

    def run(in_maps):
        concat_in = [
            jax.device_put(
                np.concatenate([np.asarray(m[nm]) for m in in_maps], axis=0), sh
            )
            for nm in in_names
        ]
        import jax.numpy as jnp

        concat_zeros = [
            jax.device_put(
                jnp.zeros((n_cores * a.shape[0], *a.shape[1:]), a.dtype), sh
            )
            for a in out_avals
        ]
        jax.block_until_ready(concat_in)
        jax.block_until_ready(concat_zeros)
        out_arrs = sharded(*concat_in, *concat_zeros)
        jax.block_until_ready(out_arrs)
        return [
            {
                nm: np.asarray(out_arrs[i]).reshape(n_cores, *out_avals[i].shape)[c]
                for i, nm in enumerate(out_names)
            }
            for c in range(n_cores)
        ]

    return run


def _build():
    nc = bacc.Bacc("TRN2", target_bir_lowering=False, debug=False, num_devices=8)

    xq_d = nc.dram_tensor("xq", [P, 2, FR], BF16, kind="ExternalInput").ap()
    xk_d = nc.dram_tensor("xk", [P, NT, SLEN], BF16, kind="ExternalInput").ap()
    xv_d = nc.dram_tensor("xv", [P, NT, SLEN], BF16, kind="ExternalInput").ap()
    wpack_d = nc.dram_tensor("wpack", [P, WLEN], BF16, kind="ExternalInput").ap()
    wps_d = nc.dram_tensor("wps", [P, WSLEN], F32, kind="ExternalInput").ap()
    zz_d = nc.dram_tensor("zz", [64, 64], BF16, kind="ExternalInput").ap()

    out_d = nc.dram_tensor("out", [P, 2, FR], F32, kind="ExternalOutput").ap()

    gpart_d = nc.dram_tensor("g_part", [P, C], F32).ap()
    gred_d = nc.dram_tensor("g_red", [P, C], F32).ap()
    warm_d = nc.dram_tensor("cc_warm", [1, 64], F32).ap()
    warm_o = nc.dram_tensor("cc_warm_o", [1, 64], F32).ap()

    from concourse.tile_rust import add_dep_helper

    with tile.TileContext(nc) as tc:
        with (
            tc.tile_pool(name="const", bufs=1) as cpool,
            tc.tile_pool(name="img", bufs=2) as ipool,
            tc.tile_pool(name="xqp", bufs=1) as xqpool,
            tc.tile_pool(name="mid", bufs=4) as mpool,
            tc.tile_pool(name="kvt", bufs=2) as tpool,
            tc.tile_pool(name="work", bufs=1) as wpool,
            tc.tile_pool(name="ps", bufs=8, space="PSUM") as psp,
        ):
            # ---- phase-A constants: K-conv prerequisites only ----
            wp = cpool.tile([P, WLEN], BF16, tag="wp")
            wps_sb = cpool.tile([P, WSLEN], F32, tag="wps")
            nc.scalar.dma_start(
                wp[:, KW_O : KW_O + 2048], wpack_d[:, KW_O : KW_O + 2048]
            )
            nc.scalar.dma_start(wp[:, ID_O:], wpack_d[:, ID_O:])
            nc.scalar.dma_start(wps_sb[:], wps_d)
            qw_v = wp[:, QW_O : QW_O + 2048].rearrange(
                "p (i d o) -> p i d o", i=2, d=4
            )
            kw_v = wp[:, KW_O : KW_O + 2048].rearrange(
                "p (i d o) -> p i d o", i=2, d=4
            )
            vw_v = wp[:, VW_O : VW_O + 2048].rearrange(
                "p (i d o) -> p i d o", i=2, d=4
            )
            fw1_v = wp[:, FW1_O : FW1_O + 512].rearrange("p (i o) -> p i o", i=2)
            fw2_v = wp[:, FW2_O : FW2_O + 512].rearrange("p (i o) -> p i o", i=2)
            id_v = wp[:, ID_O : ID_O + P]

            def sca(off):  # [P, 1] fp32 per-partition scalar view
                return wps_sb[:, off : off + 2]

            xk_sb = ipool.tile([P, NT, SLEN], BF16, tag="big")
            nc.gpsimd.collective_compute(
                "AllReduce",
                ADD,
                replica_groups=[[0, 1], [2, 3], [4, 5], [6, 7]],
                ins=[warm_d],
                outs=[warm_o],
            )
            xk_dmas = [
                nc.gpsimd.dma_start(xk_sb[:, s], xk_d[:, s])
                for s in range(NT)
            ]

            flip = [0]

            def cast_copy(dst, src):
                if flip[0] % 2:
                    nc.scalar.copy(dst, src)
                else:
                    nc.vector.tensor_copy(dst, src)
                flip[0] += 1

            def conv_strip(src_sb, w_v, bias_off, s, nm):
                cs = mpool.tile([P, 2, TW], BF16, tag="mid", name=f"cs_{nm}{s}")
                sv2 = src_sb[:, s].rearrange("p (i f) -> p i f", i=2)
                for occ in range(2):
                    ps = psp.tile([P, TW], F32, tag="ps")
                    psv = ps[:].rearrange("p (i j) -> p i j", j=NJ)
                    first = True
                    for icc in range(2):
                        sv = sv2[:, icc, :].rearrange("p (r w) -> p r w", w=W_IMG)
                        for dd in range(4):
                            di, dj = dd // 2, dd % 2
                            nc.tensor.matmul(
                                psv,
                                w_v[:, icc, dd, occ * P : (occ + 1) * P],
                                sv[:, di::2, dj::2],
                                start=first,
                                stop=(icc == 1 and dd == 3),
                            )
                            first = False
                    dsl = cs[:, occ, :]
                    if flip[0] % 2:
                        nc.scalar.activation(
                            dsl, ps[:], IDENT,
                            bias=sca(bias_off)[:, occ : occ + 1], scale=1.0,
                        )
                    else:
                        nc.vector.tensor_tensor(
                            dsl, ps[:],
                            sca(bias_off)[:, occ : occ + 1].to_broadcast([P, TW]),
                            ADD,
                        )
                    flip[0] += 1
                return cs

            # ---- K conv + transposes (kt_all holds all chunks) ----
            kt_all = wpool.tile([P, NMT, C], BF16, tag="ktall")
            for s in range(NT):
                ks = conv_strip(xk_sb, kw_v, KB_O, s, "k")
                for c in range(3):
                    t = 3 * s + c
                    for occ in range(2):
                        tp = psp.tile([P, P], BF16, tag="ps")
                        nc.tensor.transpose(
                            tp[:], ks[:, occ, c * P : (c + 1) * P], id_v
                        )
                        cast_copy(kt_all[:, t, occ * P : (occ + 1) * P], tp[:])

            # ---- phase-B loads (serialized behind the K strips) ----
            xv_sb = ipool.tile([P, NT, SLEN], BF16, tag="big")
            d = nc.sync.dma_start(
                wp[:, VW_O : VW_O + 2048], wpack_d[:, VW_O : VW_O + 2048]
            )
            add_dep_helper(d.ins, xk_dmas[-1].ins, reason="phase loads")
            xv_dmas = []
            for s in range(NT):
                d = nc.sync.dma_start(xv_sb[:, s], xv_d[:, s])
                add_dep_helper(d.ins, xk_dmas[-1].ins, reason="phase loads")
                xv_dmas.append(d)

            # ---- V conv + transposes + streamed Gram accumulation ----
            gps = [
                psp.tile([P, C], F32, tag="ps", name=f"gps{g}") for g in range(2)
            ]
            for s in range(NT):
                vs = conv_strip(xv_sb, vw_v, VB_O, s, "v")
                for c in range(3):
                    t = 3 * s + c
                    vtt = tpool.tile([P, C], BF16, tag="vtt")
                    for occ in range(2):
                        tp = psp.tile([P, P], BF16, tag="ps")
                        nc.tensor.transpose(
                            tp[:], vs[:, occ, c * P : (c + 1) * P], id_v
                        )
                        cast_copy(vtt[:, occ * P : (occ + 1) * P], tp[:])
                    for g in range(2):
                        gmm = nc.tensor.matmul(
                            gps[g][:],
                            vtt[:, g * P : (g + 1) * P],
                            kt_all[:, t, :],
                            start=(t == 0),
                            stop=(t == NMT - 1),
                            skip_group_check=True,
                        )

            # ---- Gram exchange: stage only the per-head diagonal blocks
            #      (zeros elsewhere) so the reduced result is already
            #      block-diagonal and loads back in one cast DMA ----
            gstage = wpool.tile([P, 2, P], F32, tag="gstage")
            nc.gpsimd.memzero(gstage[:])
            for g in range(2):
                for hh in range(2):
                    r0 = 64 * hh
                    nc.vector.tensor_copy(
                        gstage[r0 : r0 + 64, g, r0 : r0 + 64],
                        gps[g][r0 : r0 + 64, g * P + r0 : g * P + r0 + 64],
                    )
            nc.sync.dma_start(gpart_d, gstage[:])
            nc.gpsimd.collective_compute(
                "AllReduce",
                ADD,
                replica_groups=[[0, 1], [2, 3], [4, 5], [6, 7]],
                ins=[gpart_d],
                outs=[gred_d],
            )
            g_sb = wpool.tile([P, 2, P], BF16, tag="gsb")
            nc.gpsimd.dma_start(
                g_sb[:], gred_d.rearrange("p (g k) -> p g k", g=2)
            )

            # ---- phase-C loads (Q conv + pose prerequisites) ----
            d = nc.sync.dma_start(
                wp[:, QW_O : QW_O + 2048], wpack_d[:, QW_O : QW_O + 2048]
            )
            add_dep_helper(d.ins, xv_dmas[-1].ins, reason="phase loads")
            xq_sb = xqpool.tile([P, 2, FR], BF16, tag="xq")
            d = nc.sync.dma_start(xq_sb[:], xq_d)
            add_dep_helper(d.ins, xv_dmas[-1].ins, reason="phase loads")
            d = nc.sync.dma_start(
                wp[:, FW1_O : FW1_O + 1024], wpack_d[:, FW1_O : FW1_O + 1024]
            )
            add_dep_helper(d.ins, xv_dmas[-1].ins, reason="phase loads")

            # ---- Q conv (fills the collective latency window) ----
            q_sb = wpool.tile([P, 2, NLOC], BF16, tag="q")
            xqv = [
                xq_sb[:, icc, :].rearrange("p (r w) -> p r w", w=W_IMG)
                for icc in range(2)
            ]
            for qcc in range(2):
                for nt in range(NT):
                    ps = psp.tile([P, TW], F32, tag="ps")
                    psv = ps[:].rearrange("p (i j) -> p i j", j=NJ)
                    first = True
                    for icc in range(2):
                        for dd in range(4):
                            di, dj = dd // 2, dd % 2
                            mm = nc.tensor.matmul(
                                psv,
                                qw_v[:, icc, dd, qcc * P : (qcc + 1) * P],
                                xqv[icc][:, 16 * nt + di : 16 * nt + 16 : 2, dj::2],
                                start=first,
                                stop=(icc == 1 and dd == 3),
                            )
                            if first:
                                add_dep_helper(
                                    mm.ins, gmm.ins, sync=False,
                                    reason="pin Q conv after Gram",
                                )
                            first = False
                    if nt % 2:
                        nc.scalar.activation(
                            q_sb[:, qcc, nt * TW : (nt + 1) * TW], ps[:], IDENT,
                            bias=sca(QB_O)[:, qcc : qcc + 1], scale=1.0,
                        )
                    else:
                        nc.vector.tensor_tensor(
                            q_sb[:, qcc, nt * TW : (nt + 1) * TW], ps[:],
                            sca(QB_O)[:, qcc : qcc + 1].to_broadcast([P, TW]),
                            ADD,
                        )

            # ---- pose term: matmul + copy to staging (no z dependency) ----
            stages = []
            for oc in range(2):
                ost = ipool.tile([P, FR], BF16, tag="big", name=f"ost{oc}")
                stages.append(ost)
                for ot in range(OT):
                    ps = psp.tile([P, TW], F32, tag="ps")
                    for icc in range(2):
                        nc.tensor.matmul(
                            ps[:],
                            fw1_v[:, icc, oc * P : (oc + 1) * P],
                            xq_sb[:, icc, ot * TW : (ot + 1) * TW],
                            start=(icc == 0),
                            stop=(icc == 1),
                        )
                    if ot % 2:
                        nc.scalar.copy(ost[:, ot * TW : (ot + 1) * TW], ps[:])
                    else:
                        nc.vector.tensor_copy(
                            ost[:, ot * TW : (ot + 1) * TW], ps[:]
                        )

            # ---- W_cat^T = blockdiag(G) @ fw2'^T  (gamma/8 pre-folded) ----
            w_sb = wpool.tile([P, 2, C], BF16, tag="w")
            for g in range(2):
                psw = psp.tile([P, C], F32, tag="ps")
                nc.tensor.matmul(
                    psw[:], g_sb[:, g, :], fw2_v[:, g, :], start=True, stop=True
                )
                nc.scalar.copy(w_sb[:, g, :], psw[:])

            # ---- z'' = W_cat^T.T @ Q + fb ----
            z_sb = wpool.tile([P, 2, NLOC], F32, tag="z")
            for oc in range(2):
                for nt in range(NT):
                    ps = psp.tile([P, TW], F32, tag="ps")
                    for g in range(2):
                        nc.tensor.matmul(
                            ps[:],
                            w_sb[:, g, oc * P : (oc + 1) * P],
                            q_sb[:, g, nt * TW : (nt + 1) * TW],
                            start=(g == 0),
                            stop=(g == 1),
                        )
                    nc.scalar.activation(
                        z_sb[:, oc, nt * TW : (nt + 1) * TW], ps[:], IDENT,
                        bias=sca(FB_O)[:, oc : oc + 1], scale=1.0,
                    )

            # ---- late pass: obuf = staging + up2x(z''), then store ----
            for oc in range(2):
                ost = stages[oc]
                zv = z_sb[:, oc, :].rearrange("p (i j) -> p i j", j=NJ)
                for half in range(2):
                    obuf = mpool.tile([P, 6 * TW], F32, tag="obuf")
                    stv = ost[:, half * 6 * TW : (half + 1) * 6 * TW].rearrange(
                        "p (i ri j rj) -> p i ri j rj", i=12, ri=2, j=NJ, rj=2
                    )
                    ov = obuf[:].rearrange(
                        "p (i ri j rj) -> p i ri j rj", i=12, ri=2, j=NJ, rj=2
                    )
                    zb = zv[:, 12 * half : 12 * (half + 1), :, None].to_broadcast(
                        [P, 12, NJ, 2]
                    )
                    for ri in range(2):
                        idx = oc * 4 + half * 2 + ri
                        eng = nc.gpsimd if idx in (0, 3, 6) else nc.vector
                        eng.tensor_tensor(
                            ov[:, :, ri, :, :], stv[:, :, ri, :, :], zb, ADD
                        )
                    nc.sync.dma_start(
                        out_d[:, oc, half * 6 * TW : (half + 1) * 6 * TW],
                        obuf[:],
                    )

    nc.compile()
    return nc


def _prep_inputs(inputs):
    """Build the 8 per-core input maps (host-side shard + weight packing)."""
    import ml_dtypes

    f = np.float32
    b16 = ml_dtypes.bfloat16
    qw, qb = np.asarray(inputs["qw"], f), np.asarray(inputs["qb"], f)
    kw, kb = np.asarray(inputs["kw"], f), np.asarray(inputs["kb"], f)
    vw, vb = np.asarray(inputs["vw"], f), np.asarray(inputs["vb"], f)
    gamma = np.asarray(inputs["gamma"], f)
    fw, fb = np.asarray(inputs["fw"], f), np.asarray(inputs["fb"], f)
    pose = np.asarray(inputs["pose_enc"], f)
    app_pose = np.asarray(inputs["app_pose_enc"], f)
    app = np.asarray(inputs["app_enc"], f)

    wpack = np.zeros((P, WLEN), dtype=b16)
    wps = np.zeros((P, WSLEN), dtype=f)

    def packw(dst_off, w):
        # w [oc, ic, 2, 2] -> [p, icc, dd, oc]
        t = w.transpose(1, 2, 3, 0).reshape(2, P, 4, C).transpose(1, 0, 2, 3)
        wpack[:, dst_off : dst_off + 2048] = t.reshape(P, 2048).astype(b16)

    packw(QW_O, qw)
    packw(KW_O, kw)
    packw(VW_O, vw)
    wpack[:, FW1_O : FW1_O + 512] = (
        fw[:, :C, 0, 0].T.reshape(2, P, C).transpose(1, 0, 2).reshape(P, 512)
    ).astype(b16)
    gsc = (np.repeat(gamma.astype(np.float64), 64) / 8.0)[:, None]
    fw2s = (fw[:, C:, 0, 0].T.astype(np.float64) * gsc).astype(f)
    wpack[:, FW2_O : FW2_O + 512] = (
        fw2s.reshape(2, P, C).transpose(1, 0, 2).reshape(P, 512)
    ).astype(b16)
    wpack[:, ID_O : ID_O + P] = np.eye(P, dtype=f).astype(b16)
    wps[:, QB_O : QB_O + 2] = qb.reshape(2, P).T
    wps[:, FB_O : FB_O + 2] = fb.reshape(2, P).T
    wps[:, KB_O : KB_O + 2] = kb.reshape(2, P).T
    wps[:, VB_O : VB_O + 2] = vb.reshape(2, P).T
    zz = np.zeros((64, 64), dtype=b16)

    def shard_q(x, b, h):  # [p, icc, fr]
        halfimg = x[b, :, RH * h : RH * (h + 1), :].reshape(2, P, FR)
        return halfimg.transpose(1, 0, 2).astype(b16)

    def shard_kv(x, b, h):  # [p, strip, icc*1536]
        halfimg = x[b, :, RH * h : RH * (h + 1), :].reshape(2, P, NT, SLEN // 2)
        return halfimg.transpose(1, 2, 0, 3).reshape(P, NT, SLEN).astype(b16)

    in_maps = []
    for c in range(8):
        b, h = c // 2, c % 2
        in_maps.append({
            "xq": shard_q(pose, b, h),
            "xk": shard_kv(app_pose, b, h),
            "xv": shard_kv(app, b, h),
            "wpack": wpack,
            "wps": wps,
            "zz": zz,
        })
    return in_maps


def _get_runner():
    global _CACHED_NC, _RUNNER
    if _CACHED_NC is None:
        _CACHED_NC = _build()
    if _RUNNER is None:
        _RUNNER = _make_runner(_CACHED_NC)
    return _RUNNER


def _assemble(results):
    out = np.empty((4, C, W_IMG, W_IMG), dtype=np.float32)
    for c in range(8):
        b, h = c // 2, c % 2
        o = results[c]["out"]  # [P, 2, FR]
        out[b, :, RH * h : RH * (h + 1), :] = o.transpose(1, 0, 2).reshape(
            C, RH, W_IMG
        )
    return out


def kernel(**inputs):
    run = _get_runner()
    in_maps = _prep_inputs(inputs)
    return _assemble(run(in_maps))
